# revision 1
# baseline (speedup 1.0000x reference)
"""Longformer-with-motifs encoder on 8 trn2 NeuronCores.

Sharding: batch(2 groups of 4 cores) x Megatron tensor-parallel(4: 3 heads
each, FF/4) with 2 bf16 AllReduces per layer inside each 4-core group.

v2 changes vs baseline:
- fp8e4(DoubleRow) GEMMs for qkv/kg, v/vg, FFN wi/wo2 (weights x16 on host,
  1/16 folded into evictions).  Residual stream carried as xq = x/4 bf16 so
  the AllReduce restores full scale; xf8 = x in fp8 feeds the GEMMs.
- attention PV computed token-major (out[query, dh]) so softmax sums are a
  per-partition column: reciprocal is [128,1] (was [1,512] = 3.3us each) and
  normalization is an activation with per-partition scale.  attT transposed
  back to feature-major via PE transposes for the Wo GEMM.
- CLS global-key column folded into the span-0 mask (host side).
- weight tiles double-buffered (bufs=2) so layer l+1 DMA overlaps compute.
"""

import sys

sys.path.insert(0, "/opt/trn_rl_repo")

import numpy as np
import ml_dtypes

import concourse.bacc as bacc
import concourse.bass as bass
import concourse.tile as tile
import concourse.mybir as mybir
from concourse.bass_utils import run_bass_kernel_spmd

BF16 = mybir.dt.bfloat16
F32 = mybir.dt.float32
FP8 = mybir.dt.float8e4
bf16 = ml_dtypes.bfloat16
f8e4 = ml_dtypes.float8_e4m3

B, S, L, H, D, FF, V = 2, 1024, 12, 12, 768, 3072, 50265
DH = D // H
W1 = 256
MAXPOS = 4098
EPS = 1e-5
N_CORES = 8
TP = 4                      # tensor-parallel degree within a group
HC = H // TP                # heads per core = 3
HD = HC * DH                # 192 local head dims
FFC = FF // TP              # 768 local ff dims
KT = D // 128               # 6 k-tiles over feature dim
NSP = 2                     # two 512-token spans
SPW = 512
WS = 16.0                   # fp8 weight prescale
IWS = 1.0 / WS

# fp8 switches (fall back to bf16 per-GEMM if accuracy demands)
FP8_QKV = True
FP8_VVG = True
FP8_WI = True
FP8_WO2 = True


def _jts(sp):
    return list(range(0, 6)) if sp == 0 else list(range(2, 8))


def build_program():
    nc = bacc.Bacc("TRN2", target_bir_lowering=False, debug=False,
                   num_devices=N_CORES)

    def din(name, shape, dt=BF16):
        return nc.dram_tensor(name, shape, dt, kind="ExternalInput").ap()

    x0T_d = din("x0T", [D, S])
    wqkkg_d = din("wqkkg", [L, D, 3 * HD])
    wvvg_d = din("wvvg", [L, D, 2 * HD])
    wo_d = din("wo", [L, HD + 1, D])
    wqg_d = din("wqg", [L, D + 1, HD])
    wi_d = din("wi", [L, D, FFC])
    wo2_d = din("wo2", [L, FFC, D])
    # bias rows: [qkv 576 | vvg 384 | wo2 768] (prescaled to match evictions)
    wb_d = din("wb", [L, 1, 3 * HD + 2 * HD + D])
    # per-128-partition bias cols for gelu: [128, 6] per layer
    wib_d = din("wib", [L, 128, KT], dt=F32)
    lnc_d = din("lnc", [L + 1, D, 4], dt=F32)
    mask_d = din("mask", [12, 128, SPW], dt=FP8)
    ident_d = din("ident", [128, 128])
    motif_d = din("motif", [415, 1])
    wd_d = din("wd", [1183, D])
    wp_d = din("wp", [D + 1, 2])
    logits_d = nc.dram_tensor("logits", [2, 1], F32, kind="ExternalOutput").ap()

    ACT = mybir.ActivationFunctionType
    ALU = mybir.AluOpType
    DR = mybir.MatmulPerfMode.DoubleRow

    with tile.TileContext(nc) as tc:
        with tc.tile_pool(name="sb1", bufs=1) as p1, \
             tc.tile_pool(name="sb2", bufs=2) as p2, \
             tc.tile_pool(name="sb3", bufs=3) as p3, \
             tc.tile_pool(name="psA", bufs=2, space="PSUM") as psA, \
             tc.tile_pool(name="psS", bufs=2, space="PSUM") as psS, \
             tc.tile_pool(name="psV", bufs=2, space="PSUM") as psV, \
             tc.tile_pool(name="psR", bufs=2, space="PSUM") as psR, \
             tc.tile_pool(name="dram", bufs=2, space="DRAM") as dpool:

            # ---------------- persistent constants ----------------
            ones_row = p1.tile([1, S], BF16, tag="ones_row")
            nc.vector.memset(ones_row[:], 1.0)
            ones128f = p1.tile([128, 1], F32, tag="ones128f")
            nc.vector.memset(ones128f[:], 1.0)
            ones128 = p1.tile([128, 1], BF16, tag="ones128")
            nc.vector.memset(ones128[:], 1.0)
            eps_t = p1.tile([1, 1], F32, tag="eps_t")
            nc.vector.memset(eps_t[:], EPS)
            zero_t = p1.tile([128, 1], F32, tag="zero_t")
            nc.vector.memset(zero_t[:], 0.0)
            eps128 = p1.tile([128, 1], F32, tag="eps128")
            nc.vector.memset(eps128[:], EPS)
            ident = p1.tile([128, 128], BF16, tag="ident")
            nc.sync.dma_start(ident[:], ident_d[:])
            mask_s = p1.tile([128, 12 * SPW], FP8, tag="mask_s")
            for i in range(12):
                nc.sync.dma_start(mask_s[:, i * SPW:(i + 1) * SPW], mask_d[i])

            # activations (persistent tags)
            xq = p1.tile([128, KT, S], BF16, tag="xq")         # x/4 resid+GEMM in
            hb = p1.tile([128, KT, S], BF16, tag="hb")         # ffn hidden
            q_s = p1.tile([64, HC * S], BF16, tag="q_s")
            k_s = p1.tile([64, HC * S], BF16, tag="k_s")
            kg_s = p1.tile([64, HC * S], BF16, tag="kg_s")
            vvg_s = p1.tile([128, 8 * (HC * 65 + HD)], BF16, tag="vvg_s")
            VBLK = HC * 65 + HD                                # 387
            for tt in range(8):
                for h in range(HC):
                    nc.vector.memset(vvg_s[:, tt * VBLK + 65 * h + 64:
                                           tt * VBLK + 65 * h + 65], 1.0)
            att_tok = p1.tile([128, 8 * HD], BF16, tag="att_tok")
            att0 = p1.tile([128, S], BF16, tag="att0")         # heads 0,1 ^T
            att1 = p1.tile([65, S], BF16, tag="att1")          # head 2 + ones
            nc.vector.memset(att1[64:65, :], 1.0)
            qg_s = p1.tile([64, HC], BF16, tag="qg_s")

            def wtile(tag, cols, dt=BF16):
                return p2.tile([128, cols], dt, tag=tag, name=tag)

            def wtile3(tag, m, dt):
                return p2.tile([128, KT, m], dt, tag=tag, name=tag)

            # ---------------- helpers ----------------
            def dma_w3(t, src, m):
                for kt in range(KT):
                    nc.sync.dma_start(t[:, kt, :], src[kt * 128:(kt + 1) * 128, :])

            def gemm_f8(ps, w3, x3, mc0, mw, sp, wb_t, bc0):
                """bf16 GEMM over 6 k-tiles + bias row step (stop=True)."""
                for kt in range(KT):
                    nc.tensor.matmul(
                        ps[0:mw, :],
                        lhsT=w3[:, kt, mc0:mc0 + mw],
                        rhs=x3[:, kt, sp * SPW:(sp + 1) * SPW],
                        start=(kt == 0), stop=False)
                nc.tensor.matmul(
                    ps[0:mw, :], lhsT=wb_t[0:1, bc0 + mc0:bc0 + mc0 + mw],
                    rhs=ones_row[0:1, sp * SPW:(sp + 1) * SPW],
                    start=False, stop=True)

            def gemm_bf(ps, w2, x2, mc0, mw, sp, wb_t, bc0, mtot):
                """bf16 fallback: w2 [128, KT*mtot+...], x2 = xq"""
                for kt in range(KT):
                    nc.tensor.matmul(
                        ps[0:mw, :],
                        lhsT=w2[:, kt * mtot + mc0: kt * mtot + mc0 + mw],
                        rhs=x2[:, kt * S + sp * SPW: kt * S + (sp + 1) * SPW],
                        start=(kt == 0), stop=False)
                nc.tensor.matmul(
                    ps[0:mw, :], lhsT=wb_t[0:1, bc0 + mc0:bc0 + mc0 + mw],
                    rhs=ones_row[0:1, sp * SPW:(sp + 1) * SPW],
                    start=False, stop=True)

            def layer_norm(z_t, lnc_t, c0):
                """z_t [128, KT*S] bf16 full-scale -> xq (=x/4) and xf8 (=x)."""
                u_row = p1.tile([1, S], BF16, tag="u_row")
                w_row = p1.tile([1, S], BF16, tag="w_row")
                # token-major stats: per tok-tile sequential accumulation
                # groups; sums and sumsqs in separate psum tiles (banks)
                stat = psR.tile([128, 8], F32, tag="row")
                statq = psR.tile([128, 8], F32, tag="row")
                for sp in range(NSP):
                    for t4 in range(4):
                        tt = sp * 4 + t4
                        for kt in range(KT):
                            zsl = z_t[:, kt * S + sp * SPW + t4 * 128:
                                      kt * S + sp * SPW + (t4 + 1) * 128]
                            nc.tensor.matmul(
                                stat[:, tt:tt + 1], lhsT=zsl,
                                rhs=ones128[:],
                                start=(kt == 0), stop=(kt == KT - 1))
                        for kt in range(KT):
                            zsl = z_t[:, kt * S + sp * SPW + t4 * 128:
                                      kt * S + sp * SPW + (t4 + 1) * 128]
                            zsq = p3.tile([128, 128], BF16, tag="zsq")
                            nc.scalar.activation(zsq[:], zsl, ACT.Square,
                                                 bias=zero_t[:])
                            nc.tensor.matmul(
                                statq[:, tt:tt + 1], lhsT=zsq[:],
                                rhs=ones128[:],
                                start=(kt == 0), stop=(kt == KT - 1))
                m8 = p2.tile([128, 8], F32, tag="m8")
                nc.scalar.activation(m8[:], stat[:, 0:8], ACT.Copy, scale=1.0 / D)
                m28 = p2.tile([128, 8], F32, tag="m28")
                nc.scalar.activation(m28[:], m8[:], ACT.Square, bias=zero_t[:])
                var8 = p2.tile([128, 8], F32, tag="var8")
                nc.vector.scalar_tensor_tensor(
                    var8[:], statq[:, 0:8], 1.0 / D, m28[:], ALU.mult, ALU.subtract)
                std8 = p2.tile([128, 8], F32, tag="std8")
                nc.scalar.activation(std8[:], var8[:], ACT.Sqrt, bias=eps128[:])
                rw = p2.tile([128, 16], BF16, tag="rw")
                with nc.allow_low_precision(reason="bf16 rsqrt col"):
                    nc.vector.reciprocal(rw[:, 0:8], std8[:])
                nc.vector.scalar_tensor_tensor(
                    rw[:, 8:16], m8[:], 1.0, rw[:, 0:8], ALU.mult, ALU.mult)
                trp = psR.tile([16, 128], F32, tag="row")
                nc.tensor.matmul(trp[:], lhsT=rw[:], rhs=ident[:],
                                 start=True, stop=True)
                trs = p3.tile([16, 128], BF16, tag="trs")
                nc.vector.tensor_copy(trs[:], trp[:])
                nc.sync.dma_start(u_row[0:1, :], trs[0:8, :])
                nc.sync.dma_start(w_row[0:1, :], trs[8:16, :])
                U0 = p1.tile([128, S], BF16, tag="U0", name="U0")
                W0 = p1.tile([128, S], BF16, tag="W0", name="W0")
                for sp in range(NSP):
                    nc.gpsimd.partition_broadcast(
                        U0[:, sp * SPW:(sp + 1) * SPW],
                        u_row[0:1, sp * SPW:(sp + 1) * SPW])
                    nc.gpsimd.partition_broadcast(
                        W0[:, sp * SPW:(sp + 1) * SPW],
                        w_row[0:1, sp * SPW:(sp + 1) * SPW])
                for kt in range(KT):
                    s_col = lnc_t[:, 4 * kt + c0: 4 * kt + c0 + 1]      # s/4
                    b_col = lnc_t[:, 4 * kt + c0 + 1: 4 * kt + c0 + 2]  # b/4
                    for sp in range(NSP):
                        zsl = z_t[:, kt * S + sp * SPW: kt * S + (sp + 1) * SPW]
                        t1 = p2.tile([128, SPW], F32, tag="t1")
                        nc.vector.scalar_tensor_tensor(
                            t1[:], zsl, 1.0, U0[:, sp * SPW:(sp + 1) * SPW],
                            ALU.mult, ALU.mult)
                        u2 = p2.tile([128, SPW], F32, tag="u2")
                        nc.vector.scalar_tensor_tensor(
                            u2[:], t1[:], 1.0, W0[:, sp * SPW:(sp + 1) * SPW],
                            ALU.mult, ALU.subtract)
                        xqs = xq[:, kt, sp * SPW:(sp + 1) * SPW]
                        nc.scalar.activation(xqs, u2[:], ACT.Identity,
                                             bias=b_col, scale=s_col)

            def allreduce_z(z_loc):
                bi = dpool.tile([128, KT * S], BF16, name="ar_in")
                bo = dpool.tile([128, KT * S], BF16, name="ar_out")
                # chunk the inbound staging at eviction granularity so each
                # chunk departs as soon as its Wo/FFN eviction lands (the
                # single big DMA only started after the last eviction).
                for sp in range(NSP):
                    for mt in range(KT):
                        sl = slice(mt * S + sp * SPW, mt * S + (sp + 1) * SPW)
                        nc.sync.dma_start(bi[:, sl], z_loc[:, sl])
                nc.gpsimd.collective_compute(
                    "AllReduce", ALU.add,
                    replica_groups=[[0, 1, 2, 3], [4, 5, 6, 7]],
                    ins=[bi[:].opt()], outs=[bo[:].opt()])
                z_new = p1.tile([128, KT * S], BF16, tag="z", name="z_new")
                nc.sync.dma_start(z_new[:], bo[:])
                return z_new

            # ---------------- embeddings ----------------
            z0 = p1.tile([128, KT * S], BF16, tag="z", name="z0")
            for kt in range(KT):
                nc.sync.dma_start(z0[:, kt * S:(kt + 1) * S],
                                  x0T_d[kt * 128:(kt + 1) * 128, :])
            lnc_e = p2.tile([128, 4 * KT], F32, tag="lnc")
            for kt in range(KT):
                nc.sync.dma_start(lnc_e[:, 4 * kt:4 * kt + 4],
                                  lnc_d[L, kt * 128:(kt + 1) * 128, :])
            layer_norm(z0, lnc_e, 0)

            # ---------------- layers ----------------
            for l in range(L):
                wqkkg = wtile3("wqkkg", 3 * HD, BF16)
                dma_w3(wqkkg, wqkkg_d[l], 3 * HD)
                wvvg = wtile3("wvvg", 2 * HD, BF16)
                dma_w3(wvvg, wvvg_d[l], 2 * HD)
                wi_s = wtile3("wi_s", FFC, BF16)
                dma_w3(wi_s, wi_d[l], FFC)
                wo2_s = wtile3("wo2_s", D, BF16)
                dma_w3(wo2_s, wo2_d[l], D)
                wb_t = p1.tile([1, 3 * HD + 2 * HD + D], BF16, tag="wb_t")
                nc.sync.dma_start(wb_t[:], wb_d[l])
                wib_t = p2.tile([128, KT], F32, tag="wib_t")
                nc.sync.dma_start(wib_t[:], wib_d[l])
                wqg = wtile("wqg", 7 * HD)
                for kt in range(KT):
                    nc.sync.dma_start(wqg[:, kt * HD:(kt + 1) * HD],
                                      wqg_d[l, kt * 128:(kt + 1) * 128, :])
                nc.sync.dma_start(wqg[0:1, KT * HD:(KT + 1) * HD],
                                  wqg_d[l, D:D + 1, :])
                wo_s = wtile("wo_s", 2 * D)
                nc.sync.dma_start(wo_s[:, 0:D], wo_d[l, 0:128, :])
                nc.sync.dma_start(wo_s[0:65, D:2 * D], wo_d[l, 128:193, :])
                lnc_t = p2.tile([128, 4 * KT], F32, tag="lnc")
                for kt in range(KT):
                    nc.sync.dma_start(lnc_t[:, 4 * kt:4 * kt + 4],
                                      lnc_d[l, kt * 128:(kt + 1) * 128, :])

                # ---- qkv/kg projections: out[64m, tok], x16 -> /16 evict ----
                for sp in range(NSP):
                    for mt in range(5):
                        mw = 128 if mt < 4 else 64
                        ps = psA.tile([128, SPW], F32, tag="psA")
                        gemm_f8(ps, wqkkg, xq, mt * 128, mw, sp, wb_t, 0)
                        for sub in range(2 if mt < 4 else 1):
                            m = 2 * mt + sub
                            kind, h = m // 3, m % 3
                            dest = (q_s, k_s, kg_s)[kind]
                            nc.vector.tensor_copy(
                                dest[0:64, h * S + sp * SPW: h * S + (sp + 1) * SPW],
                                ps[64 * sub:64 * sub + 64, :])

                # ---- v/vg projections: out[tok, dh] ----
                for tt in range(8):
                    ps = psA.tile([128, 2 * HD], F32, tag="psA")
                    for kt in range(KT):
                        nc.tensor.matmul(
                            ps[:],
                            lhsT=xq[:, kt, tt * 128:(tt + 1) * 128],
                            rhs=wvvg[:, kt, :],
                            start=(kt == 0), stop=False)
                    nc.tensor.matmul(
                        ps[:], lhsT=ones_row[0:1, 0:128],
                        rhs=wb_t[0:1, 3 * HD:3 * HD + 2 * HD],
                        start=False, stop=True)
                    base = tt * VBLK
                    for h in range(HC):
                        nc.vector.tensor_copy(
                            vvg_s[:, base + 65 * h: base + 65 * h + 64],
                            ps[:, 64 * h:64 * h + 64])
                    nc.vector.tensor_copy(
                        vvg_s[:, base + 65 * HC: base + 65 * HC + HD],
                        ps[:, HD:2 * HD])

                # ---- global query projection qgT [192, 1] (bf16, from xq) ----
                for mt in range(2):
                    mw = 128 if mt == 0 else 64
                    ps = psR.tile([128, 1], F32, tag="row")
                    for kt in range(KT + 1):
                        kk = 128 if kt < KT else 1
                        lhsT = wqg[0:kk, kt * HD + mt * 128: kt * HD + mt * 128 + mw]
                        rhs = (xq[:, kt, 0:1] if kt < KT
                               else ones_row[0:1, 0:1])
                        nc.tensor.matmul(ps[0:mw, :], lhsT=lhsT, rhs=rhs,
                                         start=(kt == 0), stop=(kt == KT))
                    for sub in range(2 if mt == 0 else 1):
                        h = 2 * mt + sub
                        nc.vector.tensor_copy(qg_s[0:64, h:h + 1],
                                              ps[64 * sub:64 * sub + 64, :])

                # ---- attention ----
                for h in range(HC):
                    # global attention -> og_row [1, 65] (og | den)
                    sg = psS.tile([128, SPW], F32, tag="sc")
                    for jt in range(8):
                        nc.tensor.matmul(
                            sg[:, jt:jt + 1],
                            lhsT=kg_s[0:64, h * S + jt * 128: h * S + (jt + 1) * 128],
                            rhs=qg_s[0:64, h:h + 1], start=True, stop=True)
                    esg = p2.tile([128, 8], BF16, tag="esg")
                    acc = p2.tile([128, 1], F32, tag="acc_sg")
                    nc.scalar.activation(esg[:], sg[:, 0:8], ACT.Exp,
                                         bias=zero_t[:], accum_out=acc[:])
                    og = psR.tile([1, 65], F32, tag="row")
                    nc.tensor.matmul(og[0:1, 64:65], lhsT=acc[:], rhs=ones128f[:],
                                     start=True, stop=True)
                    for jt in range(8):
                        nc.tensor.matmul(
                            og[0:1, 0:64],
                            lhsT=esg[:, jt:jt + 1],
                            rhs=vvg_s[:, jt * VBLK + 65 * HC + 64 * h:
                                      jt * VBLK + 65 * HC + 64 * h + 64],
                            start=(jt == 0), stop=(jt == 7))
                    for sp in range(NSP):
                        exm6 = p1.tile([128, 6 * SPW], BF16, tag="exm6")
                        jts = _jts(sp)
                        for jj, jt in enumerate(jts):
                            sc = psS.tile([128, SPW], F32, tag="sc")
                            nc.tensor.matmul(
                                sc[:],
                                lhsT=k_s[0:64, h * S + jt * 128: h * S + (jt + 1) * 128],
                                rhs=q_s[0:64, h * S + sp * SPW: h * S + (sp + 1) * SPW],
                                start=True, stop=True)
                            ex = p2.tile([128, SPW], BF16, tag="ex")
                            nc.scalar.activation(ex[:], sc[:], ACT.Exp, bias=zero_t[:])
                            midx = 6 * sp + jj
                            nc.vector.scalar_tensor_tensor(
                                exm6[:, jj * SPW:(jj + 1) * SPW], ex[:], 1.0,
                                mask_s[:, midx * SPW:(midx + 1) * SPW],
                                ALU.mult, ALU.mult)
                        if sp == 1:
                            cex = p1.tile([1, SPW], BF16, tag="cex")
                            csc = psR.tile([1, SPW], F32, tag="row")
                            nc.tensor.matmul(
                                csc[:], lhsT=k_s[0:64, h * S: h * S + 1],
                                rhs=q_s[0:64, h * S + SPW: h * S + S],
                                start=True, stop=True)
                            nc.scalar.activation(cex[:], csc[:], ACT.Exp,
                                                 bias=zero_t[0:1, :])
                        # PV token-major: out [128q, 65] per query tile
                        for qt in range(4):
                            pv = psV.tile([128, 65], F32, tag="pv")
                            for jj in range(6):
                                nc.tensor.matmul(
                                    pv[:],
                                    lhsT=exm6[:, jj * SPW + qt * 128:
                                              jj * SPW + qt * 128 + 128],
                                    rhs=vvg_s[:, jts[jj] * VBLK + 65 * h:
                                              jts[jj] * VBLK + 65 * h + 65],
                                    start=(jj == 0),
                                    stop=(sp == 0 and jj == 5))
                            if sp == 1:
                                nc.tensor.matmul(
                                    pv[:],
                                    lhsT=cex[0:1, qt * 128:qt * 128 + 128],
                                    rhs=vvg_s[0:1, 65 * h: 65 * h + 65],
                                    start=False, stop=True)
                            if sp == 0 and qt == 0:
                                # CLS token: replace with global attention out
                                nc.vector.tensor_copy(pv[0:1, :], og[0:1, :])
                            r = p3.tile([128, 1], F32, tag="rcp")
                            nc.vector.reciprocal(r[:], pv[:, 64:65])
                            nc.scalar.activation(
                                att_tok[:, ((sp * 4 + qt) * HC + h) * 64:
                                        ((sp * 4 + qt) * HC + h) * 64 + 64],
                                pv[:, 0:64], ACT.Identity,
                                bias=zero_t[:], scale=r[:])

                # transpose att_tok -> att0/att1 (feature-major for Wo)
                for sp in range(NSP):
                    for qt in range(4):
                        base = (sp * 4 + qt) * HD
                        tr = psV.tile([128, 256], F32, tag="pv")
                        nc.tensor.matmul(tr[:, 0:128],
                                         lhsT=att_tok[:, base:base + 128],
                                         rhs=ident[:], start=True, stop=True)
                        nc.tensor.matmul(tr[0:64, 128:256],
                                         lhsT=att_tok[:, base + 128:base + 192],
                                         rhs=ident[:], start=True, stop=True)
                        nc.vector.tensor_copy(
                            att0[:, sp * SPW + qt * 128: sp * SPW + qt * 128 + 128],
                            tr[:, 0:128])
                        nc.vector.tensor_copy(
                            att1[0:64, sp * SPW + qt * 128: sp * SPW + qt * 128 + 128],
                            tr[0:64, 128:256])

                # ---- output projection + residual ----
                z_loc = p1.tile([128, KT * S], BF16, tag="z", name="z_loc")
                for sp in range(NSP):
                    for mt in range(KT):
                        ps = psA.tile([128, SPW], F32, tag="psA")
                        nc.tensor.matmul(
                            ps[:], lhsT=wo_s[:, mt * 128:(mt + 1) * 128],
                            rhs=att0[:, sp * SPW:(sp + 1) * SPW],
                            start=True, stop=False)
                        nc.tensor.matmul(
                            ps[:], lhsT=wo_s[0:65, D + mt * 128: D + (mt + 1) * 128],
                            rhs=att1[:, sp * SPW:(sp + 1) * SPW],
                            start=False, stop=True)
                        nc.vector.scalar_tensor_tensor(
                            z_loc[:, mt * S + sp * SPW: mt * S + (sp + 1) * SPW],
                            ps[:], 1.0,
                            xq[:, mt, sp * SPW:(sp + 1) * SPW],
                            ALU.mult, ALU.add)
                z1 = allreduce_z(z_loc)
                layer_norm(z1, lnc_t, 0)

                # ---- FFN ----
                for sp in range(NSP):
                    for mt in range(KT):
                        ps = psA.tile([128, SPW], F32, tag="psA")
                        for kt in range(KT):
                            nc.tensor.matmul(
                                ps[:],
                                lhsT=wi_s[:, kt, mt * 128:(mt + 1) * 128],
                                rhs=xq[:, kt, sp * SPW:(sp + 1) * SPW],
                                start=(kt == 0), stop=(kt == KT - 1))
                        nc.scalar.activation(
                            hb[:, mt, sp * SPW:(sp + 1) * SPW],
                            ps[:], ACT.Gelu, bias=wib_t[:, mt:mt + 1])
                z_loc2 = p1.tile([128, KT * S], BF16, tag="z", name="z_loc2")
                for sp in range(NSP):
                    for mt in range(KT):
                        ps = psA.tile([128, SPW], F32, tag="psA")
                        for kt in range(KT):
                            nc.tensor.matmul(
                                ps[:],
                                lhsT=wo2_s[:, kt, mt * 128:(mt + 1) * 128],
                                rhs=hb[:, kt, sp * SPW:(sp + 1) * SPW],
                                start=(kt == 0), stop=False)
                        nc.tensor.matmul(
                            ps[:], lhsT=wb_t[0:1, 5 * HD + mt * 128:
                                             5 * HD + (mt + 1) * 128],
                            rhs=ones_row[0:1, sp * SPW:(sp + 1) * SPW],
                            start=False, stop=True)
                        nc.vector.scalar_tensor_tensor(
                            z_loc2[:, mt * S + sp * SPW: mt * S + (sp + 1) * SPW],
                            ps[:], 1.0,
                            xq[:, mt, sp * SPW:(sp + 1) * SPW],
                            ALU.mult, ALU.add)
                z2 = allreduce_z(z_loc2)
                layer_norm(z2, lnc_t, 2)

            # ---------------- classification head ----------------
            wp_s = p1.tile([128, 14], BF16, tag="wp_s")
            for kt in range(6):
                nc.sync.dma_start(wp_s[:, 2 * kt:2 * kt + 2],
                                  wp_d[kt * 128:(kt + 1) * 128, :])
            nc.sync.dma_start(wp_s[0:1, 12:14], wp_d[768:769, :])
            mot_s = p1.tile([128, 4], BF16, tag="mot_s")
            for c in range(4):
                sz = 128 if c < 3 else 31
                nc.sync.dma_start(mot_s[0:sz, c:c + 1],
                                  motif_d[128 * c:128 * c + sz, :])

            wd_s = p1.tile([128, 10 * D], BF16, tag="wd_s")
            for kt in range(9):
                nc.sync.dma_start(wd_s[:, kt * D:(kt + 1) * D],
                                  wd_d[kt * 128:(kt + 1) * 128, :])
            nc.sync.dma_start(wd_s[0:31, 9 * D:10 * D], wd_d[9 * 128:1183, :])
            ty = p1.tile([128, KT], BF16, tag="ty")
            for mt in range(KT):
                ps = psR.tile([128, 1], F32, tag="row")
                for kt in range(10):
                    kk = 128 if kt < 9 else 31
                    lhsT = wd_s[0:kk, kt * D + mt * 128: kt * D + (mt + 1) * 128]
                    rhs = (xq[:, kt, 0:1] if kt < KT
                           else mot_s[0:kk, kt - KT: kt - KT + 1])
                    nc.tensor.matmul(ps[:], lhsT=lhsT, rhs=rhs,
                                     start=(kt == 0), stop=(kt == 9))
                nc.scalar.activation(ty[:, mt:mt + 1], ps[:], ACT.Tanh,
                                     bias=zero_t[:])
            lg_ps = psR.tile([2, 1], F32, tag="row")
            for kt in range(7):
                kk = 128 if kt < 6 else 1
                lhsT = wp_s[0:kk, 2 * kt:2 * kt + 2]
                rhs = ty[:, kt:kt + 1] if kt < 6 else ones_row[0:1, 0:1]
                nc.tensor.matmul(lg_ps[:], lhsT=lhsT, rhs=rhs,
                                 start=(kt == 0), stop=(kt == 6))
            lg_s = p1.tile([2, 1], F32, tag="lg_s")
            nc.vector.tensor_copy(lg_s[:], lg_ps[:])
            nc.sync.dma_start(logits_d[:], lg_s[:])

    nc.compile()
    return nc


def prep_inputs(inputs):
    """host-side sharding: returns in_maps for the 8 cores"""
    f32 = np.float32
    ids = np.asarray(inputs["input_ids"])
    motif = np.asarray(inputs["motif_dist"], f32)
    emb_word = np.asarray(inputs["emb_word"], f32)
    emb_pos = np.asarray(inputs["emb_pos"], f32)
    emb_type = np.asarray(inputs["emb_type"], f32)
    g = {k: np.asarray(inputs[k], f32) for k in
         ("Wq", "bq", "Wk", "bk", "Wv", "bv", "Wqg", "bqg", "Wkg", "bkg",
          "Wvg", "bvg", "Wo", "bo", "ln1_s", "ln1_b", "Wi", "bi", "Wo2",
          "bo2", "ln2_s", "ln2_b", "emb_ln_s", "emb_ln_b",
          "head_Wd", "head_bd", "head_Wp", "head_bp")}
    scale = 1.0 / np.sqrt(DH)

    def to8(x):
        return np.clip(x, -240.0, 240.0).astype(f8e4)

    # masks [12, 128, 512]; span-0 jj=0 tile carries the CLS (j==0) column
    mask = np.zeros((12, 128, SPW), f32)
    for sp in range(NSP):
        for jj, jt in enumerate(_jts(sp)):
            j = 128 * jt + np.arange(128)[:, None]
            i = SPW * sp + np.arange(SPW)[None, :]
            m = (np.abs(j - i) <= W1) & (j != 0)
            if sp == 0 and jj == 0:
                m = m | (j == 0)
            mask[6 * sp + jj] = m.astype(f32)

    # lnc [13, 768, 4]: (s/4, b/4, s2/4, b2/4); [12] = embedding LN
    lnc = np.zeros((L + 1, D, 4), f32)
    for l in range(L):
        lnc[l, :, 0] = g["ln1_s"][l] * 0.25
        lnc[l, :, 1] = g["ln1_b"][l] * 0.25
        lnc[l, :, 2] = g["ln2_s"][l] * 0.25
        lnc[l, :, 3] = g["ln2_b"][l] * 0.25
    lnc[L, :, 0] = g["emb_ln_s"] * 0.25
    lnc[L, :, 1] = g["emb_ln_b"] * 0.25

    # head: rows 0..767 consume xq (=x/4) -> x4; motif/bias rows unscaled
    wd_aug = np.concatenate([g["head_Wd"][:D] * 4.0, g["head_Wd"][D:],
                             g["head_bd"][None, :]], 0)     # [1183, 768]
    wp_aug = np.concatenate([g["head_Wp"], g["head_bp"][None, :]], 0)

    in_maps = []
    for core in range(N_CORES):
        b, r = core // TP, core % TP
        hs = slice(HD * r, HD * (r + 1))
        fs = slice(FFC * r, FFC * (r + 1))
        x0 = emb_word[ids[b]] + emb_pos[2:2 + S] + emb_type[0]
        wqkkg = np.zeros((L, D, 3 * HD), f32)
        wvvg = np.zeros((L, D, 2 * HD), f32)
        wo = np.zeros((L, HD + 1, D), f32)
        wqg = np.zeros((L, D + 1, HD), f32)
        wi = np.zeros((L, D, FFC), f32)
        wo2 = np.zeros((L, FFC, D), f32)
        wb = np.zeros((L, 1, 5 * HD + D), f32)
        wib = np.zeros((L, 128, KT), f32)
        for l in range(L):
            wqkkg[l] = np.concatenate(
                [g["Wq"][l][:, hs] * scale, g["Wk"][l][:, hs],
                 g["Wkg"][l][:, hs]], 1) * 4.0
            wb[l, 0, 0:3 * HD] = np.concatenate(
                [g["bq"][l][hs] * scale, g["bk"][l][hs], g["bkg"][l][hs]])
            wvvg[l] = np.concatenate(
                [g["Wv"][l][:, hs], g["Wvg"][l][:, hs]], 1) * 4.0
            wb[l, 0, 3 * HD:5 * HD] = np.concatenate(
                [g["bv"][l][hs], g["bvg"][l][hs]])
            wo[l, :HD] = g["Wo"][l][hs, :]
            wo[l, HD] = g["bo"][l] * 0.25
            wqg[l, :D] = g["Wqg"][l][:, hs] * scale * 4.0
            wqg[l, D] = g["bqg"][l][hs] * scale
            wi[l] = g["Wi"][l][:, fs] * 4.0
            wib[l] = g["bi"][l][fs].reshape(KT, 128).T
            wo2[l] = g["Wo2"][l][fs, :]
            wb[l, 0, 5 * HD:] = g["bo2"][l] * 0.25
        d = {
            "x0T": x0.T.copy().astype(bf16),
            "wqkkg": wqkkg.astype(bf16),
            "wvvg": wvvg.astype(bf16), "wi": wi.astype(bf16),
            "wo2": wo2.astype(bf16),
            "wo": wo.astype(bf16), "wqg": wqg.astype(bf16),
            "wb": wb.astype(bf16), "wib": wib.astype(f32),
            "lnc": lnc.astype(f32),
            "mask": to8(mask),
            "ident": np.eye(128, dtype=f32).astype(bf16),
            "motif": np.concatenate([motif[b], [1.0]]).astype(f32)[:, None].astype(bf16),
            "wd": wd_aug.astype(bf16), "wp": wp_aug.astype(bf16),
        }
        in_maps.append(d)
    return in_maps


_NC_CACHE = {}


def run(inputs, trace=False):
    if "nc" not in _NC_CACHE:
        _NC_CACHE["nc"] = build_program()
    nc = _NC_CACHE["nc"]
    in_maps = prep_inputs(inputs)
    res = run_bass_kernel_spmd(nc, in_maps, core_ids=list(range(N_CORES)),
                               trace=trace)
    out = np.stack([res.results[0]["logits"][:, 0],
                    res.results[TP]["logits"][:, 0]]).astype(np.float32)
    return out, res


def kernel(**inputs) -> np.ndarray:
    out, _ = run(inputs)
    return out

